# revision 19
# baseline (speedup 1.0000x reference)
"""CARAFE content-aware upsampling kernel for Trainium2 (8 NeuronCores).

Problem: x(4,256,64,64) -> 1x1 down-conv(64ch) -> 3x3 enc-conv(100ch) ->
softmax over 25 reassembly taps -> content-aware reassembly + pixel shuffle
(x2) -> 1x1 out-conv(256ch).  Output (4,256,128,128).

Sharding: data-parallel over (batch n, H-half) = 8 shards; each core computes
32 output rows (64 upsampled rows) of one image.

All matmul operands are 16-bit (1 PE cycle/row; fp32 runs 4 on the PE).  The
PE row count and the gpsimd scatter both bound the kernel, so the reassembly
packs the 5 dy-taps into the contraction dimension using w-QUARTERS:

  A) t = W_down@x + b_down         bf16 (64, 34, 68)
  B) e = conv3x3(t) + b_enc        9 taps as (dy01-pair, dy2+bias) matmuls
     against a row-shifted copy of t -> softmax over 25 taps -> kern fp16
  C) y = W_out@x                   low-res (the 1x1 out-conv commutes with
     the reassembly; softmax weights sum to 1 so b_out is added on the host),
     yT[w', row, c] fp16
  D) out[pix64, c](h, q) = sum_{(dy,u)} B5q[(dy,u), pix] * Y5q[(dy,u), h, c]
     per w-quarter q: u in [0,20) covers a 16+4 window, partitions
     (dy*20+u) = 100 of 128 -> ONE 256-row matmul per (h, q).
     Y5q = 20 shifted DMA copies of yT.  B5q built by gpsimd.local_scatter
     from S5q; S5q (j-shifted, dy-replicated kern rows) is built with 40
     one-hot shift matmuls (the PE moves data across partitions).
     4 quarter-MMs accumulate disjoint slices of one [128,2,256] PSUM tile;
     one copy + one DMA per output row.
"""
import sys

for _p in ("/opt/trn_rl_repo",):
    if _p not in sys.path:
        sys.path.insert(0, _p)

import numpy as np
import ml_dtypes

BF16 = ml_dtypes.bfloat16
F16 = np.float16

N, C, H, W = 4, 256, 64, 64
D, KUP = 2, 5
CM, E, OC = 64, 100, 256
HH = 32          # output rows per core
RS = HH + 4      # x slab rows (2-halo each side)
TR = HH + 2      # t rows (1-halo each side)
WP = W + 4       # padded width
NH = 8           # rows per scatter call
SCH = 112        # scatter channels (100 partitions padded to %16)

_CACHE = {}


def _scatter_table() -> np.ndarray:
    """idx[part, hi, j, dy', p] -> hi*64 + pix, or -1.

    Partition part = dy*20 + u holds S5 values kern[w_row = 16q+u+j-4, ch]
    for all (j, dy', p).  Element (j, dy', p) lands at pix = i*32 + wl*2 + jj
    (wl = u-4+j) iff dy' == dy and wl in [0,16).  Same table for every q.
    """
    t = np.full((SCH, NH, 5, 5, 4), -1, np.int16)
    for dy in range(5):
        for u in range(20):
            part = dy * 20 + u
            for hi in range(NH):
                for j in range(5):
                    wl = u - 4 + j
                    if not (0 <= wl < 16):
                        continue
                    for p in range(4):
                        i, jj = p // 2, p % 2
                        t[part, hi, j, dy, p] = hi * 64 + i * 32 + wl * 2 + jj
    return t.reshape(SCH, NH * 100)


def _shift_mats() -> np.ndarray:
    """sh[128, q, j, r, 100] one-hot: col dy*20+u hot at row r*64 + w_row,
    w_row = 16q + u + j - 4 (when in [0,64)); dy-replicated columns."""
    m = np.zeros((128, 4, 5, 2, 100), F16)
    for q in range(4):
        for j in range(5):
            for r in range(2):
                for dy in range(5):
                    for u in range(20):
                        w_row = 16 * q + u + j - 4
                        if 0 <= w_row < 64:
                            m[r * 64 + w_row, q, j, r, dy * 20 + u] = 1.0
    return m


def _build_program():
    if "nc" in _CACHE:
        return _CACHE["nc"]

    import concourse.bacc as bacc
    import concourse.mybir as mybir
    import concourse.tile as tile
    from concourse import bass

    F32, FP16, B16, I16 = (mybir.dt.float32, mybir.dt.float16,
                           mybir.dt.bfloat16, mybir.dt.int16)
    PSUM = bass.MemorySpace.PSUM

    nc = bacc.Bacc("TRN2", target_bir_lowering=False, debug=False, num_devices=8)

    xs_d = nc.dram_tensor("xs", [2, 128, RS, WP], B16, kind="ExternalInput")
    wd_d = nc.dram_tensor("wd", [2, 128, CM], B16, kind="ExternalInput")
    bd_d = nc.dram_tensor("bd", [1, CM], B16, kind="ExternalInput")
    we2_d = nc.dram_tensor("we2", [128, 3, E], B16, kind="ExternalInput")
    we1_d = nc.dram_tensor("we1", [CM + 1, 3, E], B16, kind="ExternalInput")
    wo_d = nc.dram_tensor("wo", [2, 128, OC], B16, kind="ExternalInput")
    vm_d = nc.dram_tensor("vm", [1, RS, WP], B16, kind="ExternalInput")
    id_d = nc.dram_tensor("idt", [128, 128], B16, kind="ExternalInput")
    si_d = nc.dram_tensor("six", [SCH, NH * 100], I16, kind="ExternalInput")
    sh_d = nc.dram_tensor("shm", [128, 4, 5, 2, 100], FP16, kind="ExternalInput")
    out_d = nc.dram_tensor("out", [HH, 128, 2, OC], F32, kind="ExternalOutput")

    with tile.TileContext(nc) as tc:
        with (
            tc.tile_pool(name="const", bufs=1) as cp,
            tc.tile_pool(name="esb", bufs=2) as ep_sb,
            tc.tile_pool(name="sm", bufs=2) as smp,
            tc.tile_pool(name="ro", bufs=6) as rop,
        ):
            xs0 = cp.tile([128, RS, WP], B16, tag="xs0")
            xs1 = cp.tile([128, RS, WP], B16, tag="xs1")
            wd0 = cp.tile([128, CM], B16, tag="wd0")
            wd1 = cp.tile([128, CM], B16, tag="wd1")
            bd_t = cp.tile([1, CM], B16, tag="bd")
            we2_t = cp.tile([128, 3, E], B16, tag="we2")
            we1_t = cp.tile([CM + 1, 3, E], B16, tag="we1")
            wo0 = cp.tile([128, OC], B16, tag="wo0")
            wo1 = cp.tile([128, OC], B16, tag="wo1")
            vm_t = cp.tile([1, RS, WP], B16, tag="vm")
            id_t = cp.tile([128, 128], B16, tag="idt")
            si_t = cp.tile([SCH, NH * 100], I16, tag="six")
            sh_t = cp.tile([128, 4, 5, 2, 100], FP16, tag="shm")
            t_t = cp.tile([CM + 1, TR, WP], B16, tag="t")
            t2_t = cp.tile([128, TR - 1, WP], B16, tag="t2")
            kern = cp.tile([128, 16, E], FP16, tag="kern")
            yT = cp.tile([WP, RS, OC], FP16, tag="yT")
            s5 = [cp.tile([SCH, HH, 5, 5, 4], FP16, tag=f"s5_{q}", name=f"s5_{q}")
                  for q in range(4)]
            b5 = [cp.tile([SCH, HH, 64], FP16, tag=f"b5_{q}", name=f"b5_{q}")
                  for q in range(4)]
            y5 = [cp.tile([SCH, HH, OC], FP16, tag=f"y5_{q}", name=f"y5_{q}")
                  for q in range(4)]

            nc.sync.dma_start(xs0[:, 0:18, :], xs_d[0][:, 0:18, :])
            nc.sync.dma_start(wd0[:], wd_d[0])
            nc.sync.dma_start(xs1[:, 0:18, :], xs_d[1][:, 0:18, :])
            nc.sync.dma_start(wd1[:], wd_d[1])
            nc.sync.dma_start(bd_t[:], bd_d[:])
            nc.sync.dma_start(vm_t[:], vm_d[:])
            nc.scalar.dma_start(we2_t[:], we2_d[:])
            nc.scalar.dma_start(we1_t[:], we1_d[:])
            nc.scalar.dma_start(id_t[:], id_d[:])
            nc.scalar.dma_start(xs0[:, 18:RS, :], xs_d[0][:, 18:RS, :])
            nc.scalar.dma_start(xs1[:, 18:RS, :], xs_d[1][:, 18:RS, :])
            nc.scalar.dma_start(wo0[:], wo_d[0])
            nc.scalar.dma_start(wo1[:], wo_d[1])
            nc.scalar.dma_start(si_t[:], si_d[:])
            nc.scalar.dma_start(sh_t[:], sh_d[:])
            nc.vector.memset(t_t[CM : CM + 1, :, :], 1.0)

            # ---- phase A: t = W_down @ x + b_down (masked) ----
            with tc.tile_pool(name="tp", bufs=2, space=PSUM) as tpp:
                r0 = 0
                while r0 < TR:
                    nr = min(7, TR - r0)
                    tp = tpp.tile([CM, nr, WP], F32, tag="tp")
                    nc.tensor.matmul(tp[:], wd0[:], xs0[:, 1 + r0 : 1 + r0 + nr, :],
                                     start=True, stop=False)
                    nc.tensor.matmul(tp[:], wd1[:], xs1[:, 1 + r0 : 1 + r0 + nr, :],
                                     start=False, stop=False)
                    nc.tensor.matmul(tp[:], bd_t[:], vm_t[:, 1 + r0 : 1 + r0 + nr, :],
                                     start=False, stop=True)
                    nc.vector.tensor_copy(t_t[0:CM, r0 : r0 + nr, :], tp[:])
                    r0 += nr
            # row-shifted copy for the dy01 pair matmuls
            nc.sync.dma_start(t2_t[0:CM, :, :], t_t[0:CM, 0 : TR - 1, :])
            nc.sync.dma_start(t2_t[CM:128, :, :], t_t[0:CM, 1:TR, :])

            # ---- phase B: e = conv3x3(t) + b_enc, transpose, softmax ----
            with (
                tc.tile_pool(name="ep", bufs=2, space=PSUM) as epp,
                tc.tile_pool(name="etp", bufs=2, space=PSUM) as etpp,
            ):
                for chunk in range(4):
                    ep = epp.tile([E, 8, W], F32, tag="ep")
                    for dx in range(3):
                        nc.tensor.matmul(
                            ep[:],
                            we2_t[:, dx, :],
                            t2_t[:, 8 * chunk : 8 * chunk + 8, 1 + dx : 1 + dx + W],
                            start=(dx == 0), stop=False,
                        )
                    for dx in range(3):
                        nc.tensor.matmul(
                            ep[:],
                            we1_t[:, dx, :],
                            t_t[:, 8 * chunk + 2 : 8 * chunk + 10, 1 + dx : 1 + dx + W],
                            start=False, stop=(dx == 2),
                        )
                    es = ep_sb.tile([E, 8, W], B16, tag="es")
                    nc.vector.tensor_copy(es[:], ep[:])
                    for s in range(4):
                        etp = etpp.tile([128, E], B16, tag="etp")
                        nc.tensor.transpose(etp[:], es[:, 2 * s : 2 * s + 2, :],
                                            id_t[0:E, 0:E])
                        slot = kern[:, 4 * chunk + s, :]
                        nc.scalar.activation(slot, etp[:],
                                             mybir.ActivationFunctionType.Exp)
                        kv = slot.rearrange("p (k q) -> p q k", q=4)
                        ssum = smp.tile([128, 4, 1], F32, tag="ssum")
                        nc.vector.tensor_reduce(ssum[:], kv, mybir.AxisListType.X,
                                                mybir.AluOpType.add)
                        rinv = smp.tile([128, 4, 1], F32, tag="rinv")
                        nc.vector.reciprocal(rinv[:], ssum[:])
                        nc.gpsimd.tensor_tensor(kv, kv, rinv[:].to_broadcast([128, 4, 25]),
                                                mybir.AluOpType.mult)

            # ---- S5 build (40 one-hot shift matmuls) + B5 scatter ----
            kern_v = kern[:].rearrange("p hp (dy dxi q) -> p hp dxi dy q",
                                       dy=5, dxi=5, q=4)
            with tc.tile_pool(name="s5p", bufs=3, space=PSUM) as s5pp:
                for q in range(4):
                    s5v = s5[q][:].rearrange("c (hp r) j d e -> c hp r j d e", r=2)
                    for j in range(5):
                        for r in range(2):
                            sp = s5pp.tile([100, 16, 5, 4], F32, tag="s5p")
                            nc.tensor.matmul(sp[:], sh_t[:, q, j, r, :],
                                             kern_v[:, :, 4 - j, :, :],
                                             start=True, stop=True)
                            if (j * 2 + r) % 2 == 0:
                                nc.vector.tensor_copy(s5v[0:100, :, r, j, :, :], sp[:])
                            else:
                                nc.scalar.copy(s5v[0:100, :, r, j, :, :], sp[:])
                for hb in range(HH // NH):
                    for q in range(4):
                        nc.gpsimd.local_scatter(
                            b5[q][:, hb * NH : (hb + 1) * NH, :],
                            s5[q][:, hb * NH : (hb + 1) * NH, :, :, :],
                            si_t[:],
                            channels=SCH, num_elems=NH * 64, num_idxs=NH * 100)

            # ---- phase C: y = W_out @ x (no bias; added on host) ----
            with tc.tile_pool(name="yp", bufs=2, space=PSUM) as ypp:
                for r in range(RS):
                    yp = ypp.tile([WP, OC], F32, tag="yp")
                    nc.tensor.matmul(yp[:], xs0[:, r, :], wo0[:], start=True, stop=False)
                    nc.tensor.matmul(yp[:], xs1[:, r, :], wo1[:], start=False, stop=True)
                    nc.vector.tensor_copy(yT[:, r, :], yp[:])

            # ---- Y5: shifted copies of yT rows (h-halves for pipelining) ----
            nd = 0
            for q in range(4):
                for dy in range(5):
                    (nc.sync if nd % 2 == 0 else nc.scalar).dma_start(
                        y5[q][dy * 20 : dy * 20 + 20, :, :],
                        yT[16 * q : 16 * q + 20, dy : dy + HH, :])
                    nd += 1

            # ---- phase D: 4 quarter-matmuls -> one PSUM tile per row ----
            with tc.tile_pool(name="rp", bufs=6, space=PSUM) as rpp:
                for h in range(HH):
                    rp = rpp.tile([128, 2, OC], F32, tag="rp")
                    for q in range(4):
                        nc.tensor.matmul(rp[64 * (q % 2) : 64 * (q % 2) + 64,
                                            q // 2, :],
                                         b5[q][0:100, h, :],
                                         y5[q][0:100, h, :],
                                         start=True, stop=True)
                    ro = rop.tile([128, 2, OC], F32, tag="ro")
                    if h % 2 == 0:
                        nc.vector.tensor_copy(ro[:], rp[:])
                    else:
                        nc.scalar.copy(ro[:], rp[:])
                    (nc.sync if h % 2 == 0 else nc.scalar).dma_start(out_d[h], ro[:])

    nc.compile()
    _CACHE["nc"] = nc
    return nc


def _host_inputs(x, W_down, b_down, W_enc, b_enc, W_out, b_out):
    """Per-core input maps (core = 2*n + h_half)."""
    wd = np.ascontiguousarray(W_down.T.reshape(2, 128, CM)).astype(BF16)
    bd = np.ascontiguousarray(b_down[None, :]).astype(BF16)
    we2 = np.zeros((128, 3, E), np.float32)
    we1 = np.zeros((CM + 1, 3, E), np.float32)
    for dx in range(3):
        we2[0:CM, dx, :] = W_enc[:, :, 0, dx].T
        we2[CM:128, dx, :] = W_enc[:, :, 1, dx].T
        we1[0:CM, dx, :] = W_enc[:, :, 2, dx].T
    we1[CM, 1, :] = b_enc
    wo = np.ascontiguousarray(W_out.T.reshape(2, 128, OC)).astype(BF16)
    idt = np.eye(128, dtype=np.float32).astype(BF16)
    six = _scatter_table()
    shm = _shift_mats()

    in_maps = []
    for core in range(8):
        n, h0 = core // 2, (core % 2) * HH
        xs = np.zeros((C, RS, WP), np.float32)
        vm = np.zeros((1, RS, WP), np.float32)
        lo, hi = max(0, h0 - 2), min(H, h0 + HH + 2)
        xs[:, lo - (h0 - 2) : hi - (h0 - 2), 2 : 2 + W] = x[n, :, lo:hi, :]
        vm[0, lo - (h0 - 2) : hi - (h0 - 2), 2 : 2 + W] = 1.0
        in_maps.append({
            "xs": xs.reshape(2, 128, RS, WP).astype(BF16),
            "wd": wd, "bd": bd,
            "we2": we2.astype(BF16), "we1": we1.astype(BF16),
            "wo": wo,
            "vm": vm.astype(BF16), "idt": idt,
            "six": six, "shm": shm,
        })
    return in_maps


def kernel(x, W_down, b_down, W_enc, b_enc, W_out, b_out):
    from concourse.bass_utils import run_bass_kernel_spmd

    nc = _build_program()
    in_maps = _host_inputs(np.asarray(x, np.float32), np.asarray(W_down, np.float32),
                           np.asarray(b_down, np.float32), np.asarray(W_enc, np.float32),
                           np.asarray(b_enc, np.float32), np.asarray(W_out, np.float32),
                           np.asarray(b_out, np.float32))
    res = run_bass_kernel_spmd(nc, in_maps, list(range(8)))
    full = np.empty((N, C, 2 * H, 2 * W), np.float32)
    for core in range(8):
        n, half = core // 2, core % 2
        # out[h, P=(phalf, i, wl, jj), qq, c]; q = qq*2 + phalf
        # -> (c, 2h+i, 32q + 2wl + jj)
        arr = res.results[core]["out"].reshape(HH, 2, 2, 16, 2, 2, OC)
        arr = arr.transpose(6, 0, 2, 5, 1, 3, 4).reshape(OC, 2 * HH, 2 * W)
        full[n, :, half * 64 : (half + 1) * 64, :] = arr
    full += np.asarray(b_out, np.float32)[None, :, None, None]
    return full


# revision 20
# speedup vs baseline: 1.2299x; 1.2299x over previous
"""CARAFE content-aware upsampling kernel for Trainium2 (8 NeuronCores).

Problem: x(4,256,64,64) -> 1x1 down-conv(64ch) -> 3x3 enc-conv(100ch) ->
softmax over 25 reassembly taps -> content-aware reassembly + pixel shuffle
(x2) -> 1x1 out-conv(256ch).  Output (4,256,128,128).

Sharding: data-parallel over (batch n, H-half) = 8 shards; each core computes
32 output rows (64 upsampled rows) of one image.

All matmul operands are 16-bit (1 PE cycle/row; fp32 runs 4 on the PE).  The
PE row count and the gpsimd scatter both bound the kernel, so the reassembly
packs the 5 dy-taps into the contraction dimension using w-QUARTERS:

  A) t = W_down@x + b_down         bf16 (64, 34, 68)
  B) e = conv3x3(t) + b_enc        9 taps as (dy01-pair, dy2+bias) matmuls
     against a row-shifted copy of t -> softmax over 25 taps -> kern fp16
  C) y = W_out@x                   low-res (the 1x1 out-conv commutes with
     the reassembly; softmax weights sum to 1 so b_out is added on the host),
     yT[w', row, c] fp16
  D) out[pix64, c](h, q) = sum_{(dy,u)} B5q[(dy,u), pix] * Y5q[(dy,u), h, c]
     per w-quarter q: u in [0,20) covers a 16+4 window, partitions
     (dy*20+u) = 100 of 128 -> ONE 256-row matmul per (h, q).
     Y5q = 20 shifted DMA copies of yT.  B5q built by gpsimd.local_scatter
     from S5q; S5q (j-shifted, dy-replicated kern rows) is built with 40
     one-hot shift matmuls (the PE moves data across partitions).
     4 quarter-MMs accumulate disjoint slices of one [128,2,256] PSUM tile;
     one copy + one DMA per output row.
"""
import sys

for _p in ("/opt/trn_rl_repo",):
    if _p not in sys.path:
        sys.path.insert(0, _p)

import numpy as np
import ml_dtypes

BF16 = ml_dtypes.bfloat16
F16 = np.float16

N, C, H, W = 4, 256, 64, 64
D, KUP = 2, 5
CM, E, OC = 64, 100, 256
HH = 32          # output rows per core
RS = HH + 4      # x slab rows (2-halo each side)
TR = HH + 2      # t rows (1-halo each side)
WP = W + 4       # padded width
NH = 8           # rows per scatter call
SCH = 112        # scatter channels (100 partitions padded to %16)

_CACHE = {}


def _scatter_table() -> np.ndarray:
    """idx[part, hi, j, dy', p] -> hi*64 + pix, or -1.

    Partition part = dy*20 + u holds S5 values kern[w_row = 16q+u+j-4, ch]
    for all (j, dy', p).  Element (j, dy', p) lands at pix = i*32 + wl*2 + jj
    (wl = u-4+j) iff dy' == dy and wl in [0,16).  Same table for every q.
    """
    t = np.full((SCH, NH, 5, 5, 4), -1, np.int16)
    for dy in range(5):
        for u in range(20):
            part = dy * 20 + u
            for hi in range(NH):
                for j in range(5):
                    wl = u - 4 + j
                    if not (0 <= wl < 16):
                        continue
                    for p in range(4):
                        i, jj = p // 2, p % 2
                        t[part, hi, j, dy, p] = hi * 64 + i * 32 + wl * 2 + jj
    return t.reshape(SCH, NH * 100)


def _shift_mats() -> np.ndarray:
    """sh[128, q, j, r, 100] one-hot: col dy*20+u hot at row r*64 + w_row,
    w_row = 16q + u + j - 4 (when in [0,64)); dy-replicated columns."""
    m = np.zeros((128, 4, 5, 2, 100), F16)
    for q in range(4):
        for j in range(5):
            for r in range(2):
                for dy in range(5):
                    for u in range(20):
                        w_row = 16 * q + u + j - 4
                        if 0 <= w_row < 64:
                            m[r * 64 + w_row, q, j, r, dy * 20 + u] = 1.0
    return m


def _build_program():
    if "nc" in _CACHE:
        return _CACHE["nc"]

    import concourse.bacc as bacc
    import concourse.mybir as mybir
    import concourse.tile as tile
    from concourse import bass

    F32, FP16, B16, I16 = (mybir.dt.float32, mybir.dt.float16,
                           mybir.dt.bfloat16, mybir.dt.int16)
    PSUM = bass.MemorySpace.PSUM

    nc = bacc.Bacc("TRN2", target_bir_lowering=False, debug=False, num_devices=8)

    xs_d = nc.dram_tensor("xs", [2, 128, RS, WP], B16, kind="ExternalInput")
    wd_d = nc.dram_tensor("wd", [2, 128, CM], B16, kind="ExternalInput")
    bd_d = nc.dram_tensor("bd", [1, CM], B16, kind="ExternalInput")
    we2_d = nc.dram_tensor("we2", [128, 3, E], B16, kind="ExternalInput")
    we1_d = nc.dram_tensor("we1", [CM + 1, 3, E], B16, kind="ExternalInput")
    wo_d = nc.dram_tensor("wo", [2, 128, OC], B16, kind="ExternalInput")
    vm_d = nc.dram_tensor("vm", [1, RS, WP], B16, kind="ExternalInput")
    id_d = nc.dram_tensor("idt", [128, 128], B16, kind="ExternalInput")
    si_d = nc.dram_tensor("six", [SCH, NH * 100], I16, kind="ExternalInput")
    sh_d = nc.dram_tensor("shm", [128, 4, 5, 2, 100], FP16, kind="ExternalInput")
    out_d = nc.dram_tensor("out", [HH, 128, 2, OC], F32, kind="ExternalOutput")

    with tile.TileContext(nc) as tc:
        with (
            tc.tile_pool(name="const", bufs=1) as cp,
            tc.tile_pool(name="esb", bufs=2) as ep_sb,
            tc.tile_pool(name="sm", bufs=2) as smp,
            tc.tile_pool(name="ro", bufs=6) as rop,
        ):
            xs0 = cp.tile([128, RS, WP], B16, tag="xs0")
            xs1 = cp.tile([128, RS, WP], B16, tag="xs1")
            wd0 = cp.tile([128, CM], B16, tag="wd0")
            wd1 = cp.tile([128, CM], B16, tag="wd1")
            bd_t = cp.tile([1, CM], B16, tag="bd")
            we2_t = cp.tile([128, 3, E], B16, tag="we2")
            we1_t = cp.tile([CM + 1, 3, E], B16, tag="we1")
            wo0 = cp.tile([128, OC], B16, tag="wo0")
            wo1 = cp.tile([128, OC], B16, tag="wo1")
            vm_t = cp.tile([1, RS, WP], B16, tag="vm")
            id_t = cp.tile([128, 128], B16, tag="idt")
            si_t = cp.tile([SCH, NH * 100], I16, tag="six")
            sh_t = cp.tile([128, 4, 5, 2, 100], FP16, tag="shm")
            t_t = cp.tile([CM + 1, TR, WP], B16, tag="t")
            t2_t = cp.tile([128, TR - 1, WP], B16, tag="t2")
            kern = cp.tile([128, 16, E], FP16, tag="kern")
            yT = cp.tile([WP, RS, OC], FP16, tag="yT")
            s5 = [cp.tile([SCH, HH, 5, 5, 4], FP16, tag=f"s5_{q}", name=f"s5_{q}")
                  for q in range(4)]
            b5 = [cp.tile([SCH, HH, 64], FP16, tag=f"b5_{q}", name=f"b5_{q}")
                  for q in range(4)]
            y5 = [cp.tile([SCH, HH, OC], FP16, tag=f"y5_{q}", name=f"y5_{q}")
                  for q in range(4)]

            nc.sync.dma_start(xs0[:, 0:18, :], xs_d[0][:, 0:18, :])
            nc.sync.dma_start(wd0[:], wd_d[0])
            nc.sync.dma_start(xs1[:, 0:18, :], xs_d[1][:, 0:18, :])
            nc.sync.dma_start(wd1[:], wd_d[1])
            nc.sync.dma_start(bd_t[:], bd_d[:])
            nc.sync.dma_start(vm_t[:], vm_d[:])
            nc.sync.dma_start(we2_t[:], we2_d[:])
            nc.sync.dma_start(we1_t[:], we1_d[:])
            nc.sync.dma_start(id_t[:], id_d[:])
            nc.scalar.dma_start(xs0[:, 18:RS, :], xs_d[0][:, 18:RS, :])
            nc.scalar.dma_start(xs1[:, 18:RS, :], xs_d[1][:, 18:RS, :])
            nc.scalar.dma_start(wo0[:], wo_d[0])
            nc.scalar.dma_start(wo1[:], wo_d[1])
            nc.scalar.dma_start(si_t[:], si_d[:])
            nc.scalar.dma_start(sh_t[:], sh_d[:])
            nc.vector.memset(t_t[CM : CM + 1, :, :], 1.0)

            # ---- phase A: t = W_down @ x + b_down (masked) ----
            with tc.tile_pool(name="tp", bufs=2, space=PSUM) as tpp:
                r0 = 0
                while r0 < TR:
                    nr = min(7, TR - r0)
                    tp = tpp.tile([CM, nr, WP], F32, tag="tp")
                    nc.tensor.matmul(tp[:], wd0[:], xs0[:, 1 + r0 : 1 + r0 + nr, :],
                                     start=True, stop=False)
                    nc.tensor.matmul(tp[:], wd1[:], xs1[:, 1 + r0 : 1 + r0 + nr, :],
                                     start=False, stop=False)
                    nc.tensor.matmul(tp[:], bd_t[:], vm_t[:, 1 + r0 : 1 + r0 + nr, :],
                                     start=False, stop=True)
                    nc.vector.tensor_copy(t_t[0:CM, r0 : r0 + nr, :], tp[:])
                    r0 += nr
            # row-shifted copy for the dy01 pair matmuls
            nc.sync.dma_start(t2_t[0:CM, :, :], t_t[0:CM, 0 : TR - 1, :])
            nc.sync.dma_start(t2_t[CM:128, :, :], t_t[0:CM, 1:TR, :])

            # ---- phase C: y = W_out @ x (no bias; added on host) ----
            with tc.tile_pool(name="yp", bufs=2, space=PSUM) as ypp:
                for r in range(RS):
                    yp = ypp.tile([WP, OC], F32, tag="yp")
                    nc.tensor.matmul(yp[:], xs0[:, r, :], wo0[:], start=True, stop=False)
                    nc.tensor.matmul(yp[:], xs1[:, r, :], wo1[:], start=False, stop=True)
                    nc.vector.tensor_copy(yT[:, r, :], yp[:])

            # ---- Y5: shifted copies of yT rows (h-halves for pipelining) ----
            for q in range(4):
                for dy in range(5):
                    nc.sync.dma_start(
                        y5[q][dy * 20 : dy * 20 + 20, :, :],
                        yT[16 * q : 16 * q + 20, dy : dy + HH, :])

            # ---- phase B: e = conv3x3(t) + b_enc, transpose, softmax ----
            with (
                tc.tile_pool(name="ep", bufs=2, space=PSUM) as epp,
                tc.tile_pool(name="etp", bufs=2, space=PSUM) as etpp,
            ):
                for chunk in range(4):
                    ep = epp.tile([E, 8, W], F32, tag="ep")
                    for dx in range(3):
                        nc.tensor.matmul(
                            ep[:],
                            we2_t[:, dx, :],
                            t2_t[:, 8 * chunk : 8 * chunk + 8, 1 + dx : 1 + dx + W],
                            start=(dx == 0), stop=False,
                        )
                    for dx in range(3):
                        nc.tensor.matmul(
                            ep[:],
                            we1_t[:, dx, :],
                            t_t[:, 8 * chunk + 2 : 8 * chunk + 10, 1 + dx : 1 + dx + W],
                            start=False, stop=(dx == 2),
                        )
                    es = ep_sb.tile([E, 8, W], B16, tag="es")
                    nc.vector.tensor_copy(es[:], ep[:])
                    for s in range(4):
                        etp = etpp.tile([128, E], B16, tag="etp")
                        nc.tensor.transpose(etp[:], es[:, 2 * s : 2 * s + 2, :],
                                            id_t[0:E, 0:E])
                        slot = kern[:, 4 * chunk + s, :]
                        nc.scalar.activation(slot, etp[:],
                                             mybir.ActivationFunctionType.Exp)
                        kv = slot.rearrange("p (k q) -> p q k", q=4)
                        ssum = smp.tile([128, 4, 1], F32, tag="ssum")
                        nc.vector.tensor_reduce(ssum[:], kv, mybir.AxisListType.X,
                                                mybir.AluOpType.add)
                        rinv = smp.tile([128, 4, 1], F32, tag="rinv")
                        nc.vector.reciprocal(rinv[:], ssum[:])
                        nc.gpsimd.tensor_tensor(kv, kv, rinv[:].to_broadcast([128, 4, 25]),
                                                mybir.AluOpType.mult)

            # ---- S5 build (40 one-hot shift matmuls) + B5 scatter ----
            kern_v = kern[:].rearrange("p hp (dy dxi q) -> p hp dxi dy q",
                                       dy=5, dxi=5, q=4)
            with tc.tile_pool(name="s5p", bufs=3, space=PSUM) as s5pp:
                for q in range(4):
                    s5v = s5[q][:].rearrange("c (hp r) j d e -> c hp r j d e", r=2)
                    for j in range(5):
                        for r in range(2):
                            sp = s5pp.tile([100, 16, 5, 4], F32, tag="s5p")
                            nc.tensor.matmul(sp[:], sh_t[:, q, j, r, :],
                                             kern_v[:, :, 4 - j, :, :],
                                             start=True, stop=True)
                            if (j * 2 + r) % 2 == 0:
                                nc.vector.tensor_copy(s5v[0:100, :, r, j, :, :], sp[:])
                            else:
                                nc.scalar.copy(s5v[0:100, :, r, j, :, :], sp[:])
                for hb in range(HH // NH):
                    for q in range(4):
                        nc.gpsimd.local_scatter(
                            b5[q][:, hb * NH : (hb + 1) * NH, :],
                            s5[q][:, hb * NH : (hb + 1) * NH, :, :, :],
                            si_t[:],
                            channels=SCH, num_elems=NH * 64, num_idxs=NH * 100)

            # ---- phase D: 4 quarter-matmuls -> one PSUM tile per row ----
            with tc.tile_pool(name="rp", bufs=6, space=PSUM) as rpp:
                for h in range(HH):
                    rp = rpp.tile([128, 2, OC], F32, tag="rp")
                    for q in range(4):
                        nc.tensor.matmul(rp[64 * (q % 2) : 64 * (q % 2) + 64,
                                            q // 2, :],
                                         b5[q][0:100, h, :],
                                         y5[q][0:100, h, :],
                                         start=True, stop=True)
                    ro = rop.tile([128, 2, OC], F32, tag="ro")
                    if h % 2 == 0:
                        nc.vector.tensor_copy(ro[:], rp[:])
                    else:
                        nc.scalar.copy(ro[:], rp[:])
                    (nc.sync if h % 2 == 0 else nc.scalar).dma_start(out_d[h], ro[:])

    nc.compile()
    _CACHE["nc"] = nc
    return nc


def _host_inputs(x, W_down, b_down, W_enc, b_enc, W_out, b_out):
    """Per-core input maps (core = 2*n + h_half)."""
    wd = np.ascontiguousarray(W_down.T.reshape(2, 128, CM)).astype(BF16)
    bd = np.ascontiguousarray(b_down[None, :]).astype(BF16)
    we2 = np.zeros((128, 3, E), np.float32)
    we1 = np.zeros((CM + 1, 3, E), np.float32)
    for dx in range(3):
        we2[0:CM, dx, :] = W_enc[:, :, 0, dx].T
        we2[CM:128, dx, :] = W_enc[:, :, 1, dx].T
        we1[0:CM, dx, :] = W_enc[:, :, 2, dx].T
    we1[CM, 1, :] = b_enc
    wo = np.ascontiguousarray(W_out.T.reshape(2, 128, OC)).astype(BF16)
    idt = np.eye(128, dtype=np.float32).astype(BF16)
    six = _scatter_table()
    shm = _shift_mats()

    in_maps = []
    for core in range(8):
        n, h0 = core // 2, (core % 2) * HH
        xs = np.zeros((C, RS, WP), np.float32)
        vm = np.zeros((1, RS, WP), np.float32)
        lo, hi = max(0, h0 - 2), min(H, h0 + HH + 2)
        xs[:, lo - (h0 - 2) : hi - (h0 - 2), 2 : 2 + W] = x[n, :, lo:hi, :]
        vm[0, lo - (h0 - 2) : hi - (h0 - 2), 2 : 2 + W] = 1.0
        in_maps.append({
            "xs": xs.reshape(2, 128, RS, WP).astype(BF16),
            "wd": wd, "bd": bd,
            "we2": we2.astype(BF16), "we1": we1.astype(BF16),
            "wo": wo,
            "vm": vm.astype(BF16), "idt": idt,
            "six": six, "shm": shm,
        })
    return in_maps


def kernel(x, W_down, b_down, W_enc, b_enc, W_out, b_out):
    from concourse.bass_utils import run_bass_kernel_spmd

    nc = _build_program()
    in_maps = _host_inputs(np.asarray(x, np.float32), np.asarray(W_down, np.float32),
                           np.asarray(b_down, np.float32), np.asarray(W_enc, np.float32),
                           np.asarray(b_enc, np.float32), np.asarray(W_out, np.float32),
                           np.asarray(b_out, np.float32))
    res = run_bass_kernel_spmd(nc, in_maps, list(range(8)))
    full = np.empty((N, C, 2 * H, 2 * W), np.float32)
    for core in range(8):
        n, half = core // 2, core % 2
        # out[h, P=(phalf, i, wl, jj), qq, c]; q = qq*2 + phalf
        # -> (c, 2h+i, 32q + 2wl + jj)
        arr = res.results[core]["out"].reshape(HH, 2, 2, 16, 2, 2, OC)
        arr = arr.transpose(6, 0, 2, 5, 1, 3, 4).reshape(OC, 2 * HH, 2 * W)
        full[n, :, half * 64 : (half + 1) * 64, :] = arr
    full += np.asarray(b_out, np.float32)[None, :, None, None]
    return full


# revision 21
# speedup vs baseline: 1.2832x; 1.0434x over previous
"""CARAFE content-aware upsampling kernel for Trainium2 (8 NeuronCores).

Problem: x(4,256,64,64) -> 1x1 down-conv(64ch) -> 3x3 enc-conv(100ch) ->
softmax over 25 reassembly taps -> content-aware reassembly + pixel shuffle
(x2) -> 1x1 out-conv(256ch).  Output (4,256,128,128).

Sharding: data-parallel over (batch n, H-half) = 8 shards; each core computes
32 output rows (64 upsampled rows) of one image.

All matmul operands are 16-bit (1 PE cycle/row; fp32 runs 4 on the PE).  The
PE row count and the gpsimd scatter both bound the kernel, so the reassembly
packs the 5 dy-taps into the contraction dimension using w-QUARTERS:

  A) t = W_down@x + b_down         bf16 (64, 34, 68)
  B) e = conv3x3(t) + b_enc        9 taps as (dy01-pair, dy2+bias) matmuls
     against a row-shifted copy of t -> softmax over 25 taps -> kern fp16
  C) y = W_out@x                   low-res (the 1x1 out-conv commutes with
     the reassembly; softmax weights sum to 1 so b_out is added on the host),
     yT[w', row, c] fp16
  D) out[pix64, c](h, q) = sum_{(dy,u)} B5q[(dy,u), pix] * Y5q[(dy,u), h, c]
     per w-quarter q: u in [0,20) covers a 16+4 window, partitions
     (dy*20+u) = 100 of 128 -> ONE 256-row matmul per (h, q).
     Y5q = 20 shifted DMA copies of yT.  B5q built by gpsimd.local_scatter
     from S5q; S5q (j-shifted, dy-replicated kern rows) is built with 40
     one-hot shift matmuls (the PE moves data across partitions).
     4 quarter-MMs accumulate disjoint slices of one [128,2,256] PSUM tile;
     one copy + one DMA per output row.
"""
import sys

for _p in ("/opt/trn_rl_repo",):
    if _p not in sys.path:
        sys.path.insert(0, _p)

import numpy as np
import ml_dtypes

BF16 = ml_dtypes.bfloat16
F16 = np.float16

N, C, H, W = 4, 256, 64, 64
D, KUP = 2, 5
CM, E, OC = 64, 100, 256
HH = 32          # output rows per core
RS = HH + 4      # x slab rows (2-halo each side)
TR = HH + 2      # t rows (1-halo each side)
WP = W + 4       # padded width
NH = 8           # rows per scatter call
SCH = 112        # scatter channels (100 partitions padded to %16)

_CACHE = {}


def _scatter_table() -> np.ndarray:
    """idx[part, hi, j, dy', p] -> hi*64 + pix, or -1.

    Partition part = dy*20 + u holds S5 values kern[w_row = 16q+u+j-4, ch]
    for all (j, dy', p).  Element (j, dy', p) lands at pix = i*32 + wl*2 + jj
    (wl = u-4+j) iff dy' == dy and wl in [0,16).  Same table for every q.
    """
    t = np.full((SCH, NH, 5, 5, 4), -1, np.int16)
    for dy in range(5):
        for u in range(20):
            part = dy * 20 + u
            for hi in range(NH):
                for j in range(5):
                    wl = u - 4 + j
                    if not (0 <= wl < 16):
                        continue
                    for p in range(4):
                        i, jj = p // 2, p % 2
                        t[part, hi, j, dy, p] = hi * 64 + i * 32 + wl * 2 + jj
    return t.reshape(SCH, NH * 100)


def _shift_mats() -> np.ndarray:
    """sh[128, q, j, r, 100] one-hot: col dy*20+u hot at row r*64 + w_row,
    w_row = 16q + u + j - 4 (when in [0,64)); dy-replicated columns."""
    m = np.zeros((128, 4, 5, 2, 100), F16)
    for q in range(4):
        for j in range(5):
            for r in range(2):
                for dy in range(5):
                    for u in range(20):
                        w_row = 16 * q + u + j - 4
                        if 0 <= w_row < 64:
                            m[r * 64 + w_row, q, j, r, dy * 20 + u] = 1.0
    return m


def _build_program():
    if "nc" in _CACHE:
        return _CACHE["nc"]

    import concourse.bacc as bacc
    import concourse.mybir as mybir
    import concourse.tile as tile
    from concourse import bass

    F32, FP16, B16, I16 = (mybir.dt.float32, mybir.dt.float16,
                           mybir.dt.bfloat16, mybir.dt.int16)
    PSUM = bass.MemorySpace.PSUM

    nc = bacc.Bacc("TRN2", target_bir_lowering=False, debug=False, num_devices=8)

    xs_d = nc.dram_tensor("xs", [2, 128, RS, WP], B16, kind="ExternalInput")
    wd_d = nc.dram_tensor("wd", [2, 128, CM], B16, kind="ExternalInput")
    bd_d = nc.dram_tensor("bd", [1, CM], B16, kind="ExternalInput")
    we2_d = nc.dram_tensor("we2", [128, 3, E], B16, kind="ExternalInput")
    we1_d = nc.dram_tensor("we1", [CM + 1, 3, E], B16, kind="ExternalInput")
    wo_d = nc.dram_tensor("wo", [2, 128, OC], B16, kind="ExternalInput")
    vm_d = nc.dram_tensor("vm", [1, RS, WP], B16, kind="ExternalInput")
    id_d = nc.dram_tensor("idt", [128, 128], B16, kind="ExternalInput")
    si_d = nc.dram_tensor("six", [SCH, NH * 100], I16, kind="ExternalInput")
    sh_d = nc.dram_tensor("shm", [128, 4, 5, 2, 100], FP16, kind="ExternalInput")
    out_d = nc.dram_tensor("out", [HH, 128, 2, OC], F32, kind="ExternalOutput")

    with tile.TileContext(nc) as tc:
        with (
            tc.tile_pool(name="const", bufs=1) as cp,
            tc.tile_pool(name="esb", bufs=2) as ep_sb,
            tc.tile_pool(name="sm", bufs=2) as smp,
            tc.tile_pool(name="ro", bufs=6) as rop,
        ):
            xs0 = cp.tile([128, RS, WP], B16, tag="xs0")
            xs1 = cp.tile([128, RS, WP], B16, tag="xs1")
            wd0 = cp.tile([128, CM], B16, tag="wd0")
            wd1 = cp.tile([128, CM], B16, tag="wd1")
            bd_t = cp.tile([1, CM], B16, tag="bd")
            we2_t = cp.tile([128, 3, E], B16, tag="we2")
            we1_t = cp.tile([CM + 1, 3, E], B16, tag="we1")
            wo0 = cp.tile([128, OC], B16, tag="wo0")
            wo1 = cp.tile([128, OC], B16, tag="wo1")
            vm_t = cp.tile([1, RS, WP], B16, tag="vm")
            id_t = cp.tile([128, 128], B16, tag="idt")
            si_t = cp.tile([SCH, NH * 100], I16, tag="six")
            sh_t = cp.tile([128, 4, 5, 2, 100], FP16, tag="shm")
            t_t = cp.tile([CM + 1, TR, WP], B16, tag="t")
            t2_t = cp.tile([128, TR - 1, WP], B16, tag="t2")
            kern = cp.tile([128, 16, E], FP16, tag="kern")
            yT = cp.tile([WP, RS, OC], FP16, tag="yT")
            s5 = [cp.tile([SCH, HH, 5, 5, 4], FP16, tag=f"s5_{q}", name=f"s5_{q}")
                  for q in range(4)]
            b5 = [cp.tile([SCH, HH, 64], FP16, tag=f"b5_{q}", name=f"b5_{q}")
                  for q in range(4)]
            y5 = [cp.tile([SCH, HH, OC], FP16, tag=f"y5_{q}", name=f"y5_{q}")
                  for q in range(4)]

            nc.sync.dma_start(xs0[:, 0:18, :], xs_d[0][:, 0:18, :])
            nc.sync.dma_start(wd0[:], wd_d[0])
            nc.sync.dma_start(xs1[:, 0:18, :], xs_d[1][:, 0:18, :])
            nc.sync.dma_start(wd1[:], wd_d[1])
            nc.sync.dma_start(bd_t[:], bd_d[:])
            nc.sync.dma_start(vm_t[:], vm_d[:])
            nc.scalar.dma_start(we2_t[:], we2_d[:])
            nc.scalar.dma_start(we1_t[:], we1_d[:])
            nc.scalar.dma_start(id_t[:], id_d[:])
            nc.scalar.dma_start(xs0[:, 18:RS, :], xs_d[0][:, 18:RS, :])
            nc.scalar.dma_start(xs1[:, 18:RS, :], xs_d[1][:, 18:RS, :])
            nc.scalar.dma_start(wo0[:], wo_d[0])
            nc.scalar.dma_start(wo1[:], wo_d[1])
            nc.scalar.dma_start(si_t[:], si_d[:])
            nc.scalar.dma_start(sh_t[:], sh_d[:])
            nc.vector.memset(t_t[CM : CM + 1, :, :], 1.0)

            # ---- phase A: t = W_down @ x + b_down (masked) ----
            with tc.tile_pool(name="tp", bufs=2, space=PSUM) as tpp:
                r0 = 0
                while r0 < TR:
                    nr = min(7, TR - r0)
                    tp = tpp.tile([CM, nr, WP], F32, tag="tp")
                    nc.tensor.matmul(tp[:], wd0[:], xs0[:, 1 + r0 : 1 + r0 + nr, :],
                                     start=True, stop=False)
                    nc.tensor.matmul(tp[:], wd1[:], xs1[:, 1 + r0 : 1 + r0 + nr, :],
                                     start=False, stop=False)
                    nc.tensor.matmul(tp[:], bd_t[:], vm_t[:, 1 + r0 : 1 + r0 + nr, :],
                                     start=False, stop=True)
                    nc.vector.tensor_copy(t_t[0:CM, r0 : r0 + nr, :], tp[:])
                    r0 += nr
            # row-shifted copy for the dy01 pair matmuls
            nc.sync.dma_start(t2_t[0:CM, :, :], t_t[0:CM, 0 : TR - 1, :])
            nc.sync.dma_start(t2_t[CM:128, :, :], t_t[0:CM, 1:TR, :])

            # ---- phase C: y = W_out @ x (no bias; added on host) ----
            with tc.tile_pool(name="yp", bufs=2, space=PSUM) as ypp:
                for r in range(RS):
                    yp = ypp.tile([WP, OC], F32, tag="yp")
                    nc.tensor.matmul(yp[:], xs0[:, r, :], wo0[:], start=True, stop=False)
                    nc.tensor.matmul(yp[:], xs1[:, r, :], wo1[:], start=False, stop=True)
                    nc.vector.tensor_copy(yT[:, r, :], yp[:])

            # ---- Y5: shifted copies of yT rows (h-halves for pipelining) ----
            for q in range(4):
                for dy in range(5):
                    nc.sync.dma_start(
                        y5[q][dy * 20 : dy * 20 + 20, :, :],
                        yT[16 * q : 16 * q + 20, dy : dy + HH, :])

            # ---- phase B: e = conv3x3(t) + b_enc, transpose, softmax ----
            with (
                tc.tile_pool(name="ep", bufs=2, space=PSUM) as epp,
                tc.tile_pool(name="etp", bufs=2, space=PSUM) as etpp,
            ):
                for chunk in range(4):
                    ep = epp.tile([E, 8, W], F32, tag="ep")
                    for dx in range(3):
                        nc.tensor.matmul(
                            ep[:],
                            we2_t[:, dx, :],
                            t2_t[:, 8 * chunk : 8 * chunk + 8, 1 + dx : 1 + dx + W],
                            start=(dx == 0), stop=False,
                        )
                    for dx in range(3):
                        nc.tensor.matmul(
                            ep[:],
                            we1_t[:, dx, :],
                            t_t[:, 8 * chunk + 2 : 8 * chunk + 10, 1 + dx : 1 + dx + W],
                            start=False, stop=(dx == 2),
                        )
                    es = ep_sb.tile([E, 8, W], B16, tag="es")
                    nc.vector.tensor_copy(es[:], ep[:])
                    for s in range(4):
                        etp = etpp.tile([128, E], B16, tag="etp")
                        nc.tensor.transpose(etp[:], es[:, 2 * s : 2 * s + 2, :],
                                            id_t[0:E, 0:E])
                        slot = kern[:, 4 * chunk + s, :]
                        nc.scalar.activation(slot, etp[:],
                                             mybir.ActivationFunctionType.Exp)
                        kv = slot.rearrange("p (k q) -> p q k", q=4)
                        ssum = smp.tile([128, 4, 1], F32, tag="ssum")
                        nc.vector.tensor_reduce(ssum[:], kv, mybir.AxisListType.X,
                                                mybir.AluOpType.add)
                        rinv = smp.tile([128, 4, 1], F32, tag="rinv")
                        nc.vector.reciprocal(rinv[:], ssum[:])
                        nc.gpsimd.tensor_tensor(kv, kv, rinv[:].to_broadcast([128, 4, 25]),
                                                mybir.AluOpType.mult)

            # ---- S5 build (40 one-hot shift matmuls) + B5 scatter ----
            kern_v = kern[:].rearrange("p hp (dy dxi q) -> p hp dxi dy q",
                                       dy=5, dxi=5, q=4)
            with tc.tile_pool(name="s5p", bufs=3, space=PSUM) as s5pp:
                for q in range(4):
                    s5v = s5[q][:].rearrange("c (hp r) j d e -> c hp r j d e", r=2)
                    for j in range(5):
                        for r in range(2):
                            sp = s5pp.tile([100, 16, 5, 4], F32, tag="s5p")
                            nc.tensor.matmul(sp[:], sh_t[:, q, j, r, :],
                                             kern_v[:, :, 4 - j, :, :],
                                             start=True, stop=True)
                            if (j * 2 + r) % 2 == 0:
                                nc.vector.tensor_copy(s5v[0:100, :, r, j, :, :], sp[:])
                            else:
                                nc.scalar.copy(s5v[0:100, :, r, j, :, :], sp[:])
                for hb in range(HH // NH):
                    for q in range(4):
                        nc.gpsimd.local_scatter(
                            b5[q][:, hb * NH : (hb + 1) * NH, :],
                            s5[q][:, hb * NH : (hb + 1) * NH, :, :, :],
                            si_t[:],
                            channels=SCH, num_elems=NH * 64, num_idxs=NH * 100)

            # ---- phase D: 4 quarter-matmuls -> one PSUM tile per row ----
            with tc.tile_pool(name="rp", bufs=6, space=PSUM) as rpp:
                for hp in range(HH // 2):
                    ro = rop.tile([128, 2, 2, OC], F32, tag="ro")
                    for hi in range(2):
                        h = 2 * hp + hi
                        rp = rpp.tile([128, 2, OC], F32, tag="rp")
                        for q in range(4):
                            nc.tensor.matmul(rp[64 * (q % 2) : 64 * (q % 2) + 64,
                                                q // 2, :],
                                             b5[q][0:100, h, :],
                                             y5[q][0:100, h, :],
                                             start=True, stop=True)
                        if hi == 0:
                            nc.vector.tensor_copy(ro[:, 0, :, :], rp[:])
                        else:
                            nc.scalar.copy(ro[:, 1, :, :], rp[:])
                    (nc.sync if hp % 2 == 0 else nc.scalar).dma_start(
                        out_d[2 * hp : 2 * hp + 2].rearrange("h p q c -> p h q c"),
                        ro[:].rearrange("p h q c -> p h q c"))

    nc.compile()
    _CACHE["nc"] = nc
    return nc


def _host_inputs(x, W_down, b_down, W_enc, b_enc, W_out, b_out):
    """Per-core input maps (core = 2*n + h_half)."""
    wd = np.ascontiguousarray(W_down.T.reshape(2, 128, CM)).astype(BF16)
    bd = np.ascontiguousarray(b_down[None, :]).astype(BF16)
    we2 = np.zeros((128, 3, E), np.float32)
    we1 = np.zeros((CM + 1, 3, E), np.float32)
    for dx in range(3):
        we2[0:CM, dx, :] = W_enc[:, :, 0, dx].T
        we2[CM:128, dx, :] = W_enc[:, :, 1, dx].T
        we1[0:CM, dx, :] = W_enc[:, :, 2, dx].T
    we1[CM, 1, :] = b_enc
    wo = np.ascontiguousarray(W_out.T.reshape(2, 128, OC)).astype(BF16)
    idt = np.eye(128, dtype=np.float32).astype(BF16)
    six = _scatter_table()
    shm = _shift_mats()

    in_maps = []
    for core in range(8):
        n, h0 = core // 2, (core % 2) * HH
        xs = np.zeros((C, RS, WP), np.float32)
        vm = np.zeros((1, RS, WP), np.float32)
        lo, hi = max(0, h0 - 2), min(H, h0 + HH + 2)
        xs[:, lo - (h0 - 2) : hi - (h0 - 2), 2 : 2 + W] = x[n, :, lo:hi, :]
        vm[0, lo - (h0 - 2) : hi - (h0 - 2), 2 : 2 + W] = 1.0
        in_maps.append({
            "xs": xs.reshape(2, 128, RS, WP).astype(BF16),
            "wd": wd, "bd": bd,
            "we2": we2.astype(BF16), "we1": we1.astype(BF16),
            "wo": wo,
            "vm": vm.astype(BF16), "idt": idt,
            "six": six, "shm": shm,
        })
    return in_maps


def kernel(x, W_down, b_down, W_enc, b_enc, W_out, b_out):
    from concourse.bass_utils import run_bass_kernel_spmd

    nc = _build_program()
    in_maps = _host_inputs(np.asarray(x, np.float32), np.asarray(W_down, np.float32),
                           np.asarray(b_down, np.float32), np.asarray(W_enc, np.float32),
                           np.asarray(b_enc, np.float32), np.asarray(W_out, np.float32),
                           np.asarray(b_out, np.float32))
    res = run_bass_kernel_spmd(nc, in_maps, list(range(8)))
    full = np.empty((N, C, 2 * H, 2 * W), np.float32)
    for core in range(8):
        n, half = core // 2, core % 2
        # out[h, P=(phalf, i, wl, jj), qq, c]; q = qq*2 + phalf
        # -> (c, 2h+i, 32q + 2wl + jj)
        arr = res.results[core]["out"].reshape(HH, 2, 2, 16, 2, 2, OC)
        arr = arr.transpose(6, 0, 2, 5, 1, 3, 4).reshape(OC, 2 * HH, 2 * W)
        full[n, :, half * 64 : (half + 1) * 64, :] = arr
    full += np.asarray(b_out, np.float32)[None, :, None, None]
    return full
